# revision 2
# baseline (speedup 1.0000x reference)
"""Minibatch-discrimination kernel for Trainium2 (8 NeuronCores, SPMD).

Math: M = einsum('nf,fbc->nbc', x, T); d[i,j,b] = sum_c |M[i,b,c]-M[j,b,c]|;
out[i,b] = sum_j exp(-d[i,j,b]) - 1; return concat([x, out], axis=1).

Sharding: rows of the output are data-parallel across 8 cores. All cores run
the IDENTICAL program; core k receives x rotated by 512*k rows so that "its"
rows sit at local rows 0..511 (the j-sum is permutation invariant).

Per-core layout: M.T is stored as [bc=512, j=4096] in 4 partition-chunks of
128; chunk ch holds (b, c) pairs with c in {2ch, 2ch+1} (partition = b*2 +
c%2), via a host-side permutation of T's columns.

abs is not encodable on the DVE tensor_scalar path, so we use
  d[i,j,b] = 2*sum_c max(u_c, v_c) - U_i[b] - V_j[b],   U/V = sum_c M[.,b,c]
The max pass is a single-op tensor_scalar per (i, chunk, j-half):
op0=max(scalar=M.T[:, i]), eligible for the DVE's 4x fp16 mode, no broadcast.
The c-sum is a K=128 matmul against a 0/1 selector (S[p,g]=p//2==g, M=64)
accumulating over the 4 chunks in PSUM ([128=(2i,64b), 2048j] tiles); the
-V_j/2 term joins the same accumulation as one K=64 matmul per 512-j tile
(lhsT = -0.5*block-identity, rhs = precomputed V.T). ScalarE then computes
exp(-2*P + U_i) = exp(-d) with the +U_i in the per-partition activation bias,
and the j-sum via accum_out, in a single activation per PSUM tile.
"""

import os
from contextlib import ExitStack

import numpy as np

N, F, B, C = 4096, 256, 64, 8
NCORES = 8
ROWS = N // NCORES          # 512 output rows per core
BC = B * C                  # 512
NCHUNK = BC // 128          # 4 partition-chunks of M.T
NPAIR = ROWS // 2           # 256 (two i's fill one 128-partition psum tile)
JW = 2048                   # j-half width per psum tile
NJH = N // JW               # 2
JSUB = JW // 512            # 4 matmuls of 512 j per (i, chunk, j-half)

_CACHE = {}


def _build_program(n=N, rows=ROWS, jw=JW):
    import concourse.bacc as bacc
    import concourse.tile as tile
    from concourse import mybir
    from concourse._compat import get_trn_type

    f32 = mybir.dt.float32
    bf16 = mybir.dt.float16  # fp16: 8x finer mantissa than bf16, same DVE/PE rates
    Alu = mybir.AluOpType
    Act = mybir.ActivationFunctionType

    npair = rows // 2
    njh = n // jw
    jsub = jw // 512 if jw >= 512 else 1
    jmm = min(jw, 512)
    nc = bacc.Bacc(
        get_trn_type() or "TRN2",
        target_bir_lowering=False,
        debug=False,
        enable_asserts=True,
        num_devices=NCORES,
    )

    x_d = nc.dram_tensor("x_rot", [n, F], f32, kind="ExternalInput").ap()
    t_d = nc.dram_tensor("t_mat", [F, BC], f32, kind="ExternalInput").ap()
    s_d = nc.dram_tensor("s_sel", [128, B], bf16, kind="ExternalInput").ap()
    id_d = nc.dram_tensor("ident", [128, 128], f32, kind="ExternalInput").ap()
    e2_d = nc.dram_tensor("e2_neg", [64, 128], f32, kind="ExternalInput").ap()
    dw_d = nc.dram_tensor("diag_w", [32, 128], bf16, kind="ExternalInput").ap()
    oh_d = nc.dram_tensor("ohbuf", [32, 1024], bf16, kind="ExternalInput").ap()
    o_d = nc.dram_tensor("out", [rows, B], f32, kind="ExternalOutput").ap()
    u_d = nc.dram_tensor("u_scratch", [2, B, rows // 2], f32).ap()

    with tile.TileContext(nc) as tc, ExitStack() as ctx:
        singles = ctx.enter_context(tc.tile_pool(name="singles", bufs=1))
        xin = ctx.enter_context(tc.tile_pool(name="xin", bufs=4))
        psum = ctx.enter_context(tc.tile_pool(name="psum", bufs=2, space="PSUM"))
        absd_p = ctx.enter_context(tc.tile_pool(name="absd", bufs=6))
        escr_p = ctx.enter_context(tc.tile_pool(name="escr", bufs=1))

        # ---- constants -----------------------------------------------------
        s_sel = singles.tile([128, B], bf16)
        nc.sync.dma_start(out=s_sel, in_=s_d)
        ident = singles.tile([128, 128], f32)
        nc.sync.dma_start(out=ident, in_=id_d)
        e2_neg = singles.tile([64, 128], f32)
        nc.sync.dma_start(out=e2_neg, in_=e2_d)
        diag_w = singles.tile([32, 128], bf16)
        nc.sync.dma_start(out=diag_w, in_=dw_d)
        ohbuf = singles.tile([32, 1024], bf16)
        nc.sync.dma_start(out=ohbuf, in_=oh_d)

        # ---- transpose x: xT[k, n] -----------------------------------------
        KCH = F // 128  # 2
        xT = [singles.tile([128, n], f32, tag=f"xT{kc}", name=f"xT{kc}")
              for kc in range(KCH)]
        x_v = x_d.rearrange("(t p) f -> t p f", p=128)  # 32 x [128, 256]
        for t in range(n // 128):
            xt_in = xin.tile([128, F], f32, tag="xtile")
            nc.sync.dma_start(out=xt_in, in_=x_v[t])
            for kc in range(KCH):
                pt = psum.tile([128, 128], f32, tag="ps")
                nc.tensor.transpose(pt, xt_in[:, kc * 128:(kc + 1) * 128], ident)
                nc.scalar.copy(out=xT[kc][:, t * 128:(t + 1) * 128], in_=pt)

        # ---- T (already column-permuted on host) in sbuf: [k, bc] ----------
        t_sb = [singles.tile([128, BC], f32, tag=f"tsb{kc}", name=f"tsb{kc}")
                for kc in range(KCH)]
        t_v = t_d.rearrange("(kc p) q -> kc p q", p=128)
        for kc in range(KCH):
            nc.sync.dma_start(out=t_sb[kc], in_=t_v[kc])

        # ---- MT = (x @ T).T as 4 chunks [128, N]; bf16 + f32 (roundtripped)
        mt_bf = [singles.tile([128, n], bf16, tag=f"mtb{ch}", name=f"mtb{ch}")
                 for ch in range(NCHUNK)]
        mt_f32 = [singles.tile([128, n], f32, tag=f"mtf{ch}", name=f"mtf{ch}")
                  for ch in range(NCHUNK)]
        for ch in range(NCHUNK):
            for jt in range(max(1, n // 512)):
                pm = psum.tile([128, jmm], f32, tag="ps")
                for kc in range(KCH):
                    nc.tensor.matmul(
                        pm,
                        t_sb[kc][:, ch * 128:(ch + 1) * 128],
                        xT[kc][:, jt * jmm:(jt + 1) * jmm],
                        start=(kc == 0),
                        stop=(kc == KCH - 1),
                    )
                # psum -> bf16 (this rounding defines the kernel's M)
                nc.vector.tensor_copy(
                    out=mt_bf[ch][:, jt * jmm:(jt + 1) * jmm], in_=pm
                )
            # bf16 -> f32 roundtrip copy (scalar operand must be f32; equals
            # the bf16 value exactly so the self-distance is exactly 0)
            nc.scalar.copy(out=mt_f32[ch], in_=mt_bf[ch])

        # ---- VT[b, j] = sum_c M[j, b, c]; fp16 copy feeds the -V/2 psum
        # correction matmuls, fp32 slice [B, rows] roundtrips through DRAM to
        # become the per-partition +U_i activation bias
        f32r = mybir.dt.float32r
        vt32 = singles.tile([64, n], f32r)
        e2_r = singles.tile([64, 128], f32r)
        nc.scalar.copy(out=e2_r, in_=e2_neg)
        for jt in range(max(1, n // 512)):
            pv_ps = psum.tile([64, jmm], f32, tag="ps", name="pv_ps")
            for ch in range(NCHUNK):
                nc.tensor.matmul(
                    pv_ps,
                    s_sel,
                    mt_bf[ch][:, jt * jmm:(jt + 1) * jmm],
                    start=(ch == 0),
                    stop=(ch == NCHUNK - 1),
                )
            nc.scalar.copy(out=vt32[:, jt * jmm:(jt + 1) * jmm], in_=pv_ps)
        # scatter-write U so the readback is contiguous:
        # u_d[i2, b, pr] = VT32[b, 2pr+i2] = U[2pr+i2, b]
        nc.sync.dma_start(
            out=u_d.rearrange("i2 b pr -> b pr i2"),
            in_=vt32.bitcast(f32)[:, 0:rows].rearrange(
                "b (pr i2) -> b pr i2", i2=2),
        )
        # u_sb[p=(i2,b), pr] = U[2pr+i2, b]
        u_sb = singles.tile([128, npair], f32)
        nc.sync.dma_start(out=u_sb, in_=u_d.rearrange("i2 b pr -> (i2 b) pr"))

        # ---- per-(i,b) partial sums: col = pr*2 + jh; partition = i2*64 + b
        psbuf = singles.tile([128, npair * njh], f32)
        escr = escr_p.tile([128, jw], bf16)

        # ---- main loop -----------------------------------------------------
        for pr in range(npair):
            for jh in range(njh):
                dps = psum.tile([128, jw], f32, tag="ps")
                for i2 in range(2):
                    i = pr * 2 + i2
                    for ch in range(NCHUNK):
                        ad = absd_p.tile([128, jw], bf16, tag="ad")
                        nc.vector.tensor_scalar(
                            out=ad,
                            in0=mt_bf[ch][:, jh * jw:(jh + 1) * jw],
                            scalar1=mt_f32[ch][:, i:i + 1],
                            scalar2=None,
                            op0=Alu.max,
                        )
                        for js in range(jsub):
                            nc.tensor.matmul(
                                dps[i2 * 64:(i2 + 1) * 64,
                                    js * jmm:(js + 1) * jmm],
                                s_sel,
                                ad[:, js * jmm:(js + 1) * jmm],
                                start=(ch == 0),
                                stop=False,
                                skip_group_check=True,
                            )
                # -V/2 correction: one K=64 f32r matmul per 512-j tile
                for js in range(jsub):
                    nc.tensor.matmul(
                        dps[:, js * jmm:(js + 1) * jmm],
                        e2_r,
                        vt32[:, jh * jw + js * jmm:jh * jw + (js + 1) * jmm],
                        start=False,
                        stop=True,
                        skip_group_check=True,
                    )
                # kill the self term: P[p, j=i(p)] += 50 -> d += 100
                if jh == 0 and not os.environ.get("KERNEL_NO_DIAG"):
                    js0 = (2 * pr) // jmm
                    q = 2 * pr - js0 * jmm
                    nc.tensor.matmul(
                        dps[:, js0 * jmm:(js0 + 1) * jmm],
                        diag_w,
                        ohbuf[:, 510 - q:510 - q + jmm],
                        start=False,
                        stop=True,
                        skip_group_check=True,
                    )
                col = pr * njh + jh
                nc.scalar.activation(
                    out=escr,
                    in_=dps,
                    func=Act.Exp,
                    scale=-2.0,
                    bias=u_sb[:, pr:pr + 1],
                    accum_out=psbuf[:, col:col + 1],
                )

        # ---- finish: out[i, b] = (part0 - 1) + part1 -----------------------
        red = singles.tile([128, npair], f32)
        pv = psbuf.rearrange("p (c two) -> p c two", two=njh)
        if njh == 2:
            nc.vector.tensor_tensor(
                out=red, in0=pv[:, :, 0], in1=pv[:, :, 1], op=Alu.add
            )
        else:
            nc.vector.tensor_copy(out=red, in_=psbuf)
        # red[:, pr]: partition = i2*64 + b. Transpose 128-blocks so the DMA
        # descriptors are contiguous 256B runs.
        o_v = o_d.rearrange("(pr i2) b -> pr i2 b", i2=2)
        bw = min(128, npair)
        for blk in range(npair // bw):
            pt = psum.tile([bw, 128], f32, tag="ps", name="ptT")
            nc.tensor.transpose(pt, red[:, blk * bw:(blk + 1) * bw], ident)
            ot = xin.tile([bw, 128], f32, tag="otile")
            nc.scalar.copy(out=ot, in_=pt)
            ot_v = ot.rearrange("q (i2 b) -> q i2 b", i2=2)
            nc.sync.dma_start(
                out=o_v[blk * bw:(blk + 1) * bw], in_=ot_v
            )

    nc.compile()
    return nc


def _get_program():
    if "nc" not in _CACHE:
        _CACHE["nc"] = _build_program()
    return _CACHE["nc"]


def _host_consts():
    s_sel = (np.arange(128)[:, None] // 2 == np.arange(B)[None, :]).astype(
        np.float16
    )
    ident = np.eye(128, dtype=np.float32)
    e2_neg = (-0.5 * (np.arange(64)[:, None] == (np.arange(128)[None, :] % 64))
              ).astype(np.float32)
    diag_w = np.zeros((32, 128), dtype=np.float16)
    diag_w[0, :64] = 50.0
    diag_w[1, 64:] = 50.0
    ohbuf = np.zeros((32, 1024), dtype=np.float16)
    ohbuf[0, 510] = 1.0  # slice [510-q : 510-q+jmm] puts it at offset q
    ohbuf[1, 511] = 1.0  # ... and this one at q+1 (the i2=1 column)
    return s_sel, ident, e2_neg, diag_w, ohbuf


def _host_inputs(x, T):
    x = np.ascontiguousarray(x, dtype=np.float32)
    # permute T columns: q = ch*128 + b*2 + e  <->  (b, c=2ch+e)
    t_mat = np.ascontiguousarray(
        T.reshape(F, B, NCHUNK, 2).transpose(0, 2, 1, 3).reshape(F, BC),
        dtype=np.float32,
    )
    s_sel, ident, e2_neg, diag_w, ohbuf = _host_consts()
    return x, t_mat, s_sel, ident, e2_neg, diag_w, ohbuf


TRACE = bool(int(os.environ.get("KERNEL_TRACE", "0")))
LAST_RESULTS = None


def _make_ntff_hook():
    # the image's antenv lacks axon_hooks, but the injected libaxon_pjrt.so
    # carries the NTFF profile C ABI — drive it via ctypes directly
    import contextlib
    import ctypes

    so_path = "/opt/axon/libaxon_pjrt.so"
    if not os.path.exists(so_path):
        return None
    lib = ctypes.CDLL(so_path)
    if not hasattr(lib, "axon_start_nrt_profile"):
        return None
    lib.axon_start_nrt_profile.argtypes = [
        ctypes.POINTER(ctypes.c_int64),
        ctypes.c_size_t,
    ]
    lib.axon_start_nrt_profile.restype = ctypes.c_int64
    lib.axon_stop_nrt_profile.argtypes = [ctypes.c_char_p]
    lib.axon_stop_nrt_profile.restype = ctypes.c_int64

    @contextlib.contextmanager
    def _hook(output_dir, device_ids):
        import jax

        jax.devices()
        if device_ids:
            ids = (ctypes.c_int64 * len(device_ids))(*device_ids)
            rc = lib.axon_start_nrt_profile(ids, len(device_ids))
        else:
            rc = lib.axon_start_nrt_profile(None, 0)
        if rc != 0:
            raise RuntimeError(f"axon_start_nrt_profile rc={rc}")
        try:
            yield
        finally:
            n = lib.axon_stop_nrt_profile(str(output_dir).encode())
            print(f"profile: {n} file(s) written to {output_dir}")

    return _hook


def _ensure_axon_hook_stub():
    # this env's axon shim lacks antenv.axon_hooks; provide a real hook via
    # ctypes when the .so supports it, else degrade into the no-trace path
    import importlib
    import sys
    import types

    try:
        importlib.import_module("antenv.axon_hooks")
    except ModuleNotFoundError:
        stub = types.ModuleType("antenv.axon_hooks")
        stub.get_axon_ntff_profile_hook = _make_ntff_hook
        sys.modules["antenv.axon_hooks"] = stub


def kernel(x: np.ndarray, T: np.ndarray) -> np.ndarray:
    global LAST_RESULTS
    _ensure_axon_hook_stub()
    from concourse.bass_utils import run_bass_kernel_spmd

    nc = _get_program()
    x, t_mat, s_sel, ident, e2_neg, diag_w, ohbuf = _host_inputs(x, T)

    in_maps = []
    for k in range(NCORES):
        x_rot = np.roll(x, -ROWS * k, axis=0) if k else x
        in_maps.append(
            {"x_rot": x_rot, "t_mat": t_mat, "s_sel": s_sel, "ident": ident,
             "e2_neg": e2_neg, "diag_w": diag_w, "ohbuf": ohbuf}
        )

    res = run_bass_kernel_spmd(
        nc, in_maps, core_ids=list(range(NCORES)), trace=TRACE
    )
    LAST_RESULTS = res
    out_b = np.concatenate([res.results[k]["out"] for k in range(NCORES)], axis=0)
    return np.concatenate([x, out_b.astype(np.float32)], axis=1)



# revision 9
# speedup vs baseline: 1.3863x; 1.3863x over previous
"""Minibatch-discrimination kernel for Trainium2 (8 NeuronCores, SPMD), v2.

Math: M = einsum('nf,fbc->nbc', x, T); d[i,j,b] = sum_c |M[i,b,c]-M[j,b,c]|;
out[i,b] = sum_j exp(-d[i,j,b]) - 1; return concat([x, out], axis=1).

v2 exploits d(i,j)=d(j,i): core k only computes pairs against local j in
[0, 2560) (own block, gaps 1-3, gap 4) instead of [0, 4096):
  - j in [0, 512): diagonal block, upper triangle only (a step-mask matmul
    adds +25 to the psum for j<=i, killing exp); row sums cover j>i, column
    sums cover i<j. The self term is never computed, so no -1 at the end.
  - j in [512, 2048): gaps 1-3, full; row sums for own rows + column sums
    emitted as partial outputs for blocks k+1..k+3 (host adds them).
  - j in [2048, 2560): gap 4, row sums only (both endpoint cores compute
    their own rows against the partner block).
Host combines row parts + rotated column parts.

The pairwise pass is split across three engines (tunable per-window):
  - DVE tensor_scalar max (4x fp16): ad = max(mt_j, mt_i)
  - DVE scalar_tensor_tensor: acc = max(mt_j, mt_i) + ad_prev (merges two
    chunks into one matmul rhs, 2x mode)
  - ScalarE relu: R = relu(mt_j - mt_i); |a| = a + 2*relu(-a) flips that
    chunk's U_i sign in the exp bias: bias = U_maxch - U_reluch.
PE sums partition pairs via a 0/1 selector (K=128 -> M=64) into PSUM
([128=(i2,64b), jw] tiles), adds -V_j/2 via a K=64 matmul, ScalarE computes
exp(-2P + bias) with the j-sum via accum_out, and column sums accumulate
over all pr in persistent PSUM tiles via a ones-pair selector matmul over
the fp16 exp tile.
"""

import os
from contextlib import ExitStack

import numpy as np

N, F, B, C = 4096, 256, 64, 8
NCORES = 8
ROWS = N // NCORES          # 512 output rows per core
JDOM = ROWS * 5             # 2560: local j domain (diag + gaps 1-3 + gap 4)
COLW = ROWS * 4             # 2048: j range with column-sum partials
BC = B * C                  # 512
NCHUNK = BC // 128          # 4 partition-chunks of M.T
NPAIR = ROWS // 2           # 256 (two i's fill one 128-partition psum tile)
RELUCH = 3                  # chunk computed by ScalarE relu on window A1

_CACHE = {}


def _build_program():
    import concourse.bacc as bacc
    import concourse.tile as tile
    from concourse import mybir
    from concourse._compat import get_trn_type

    f32 = mybir.dt.float32
    f32r = mybir.dt.float32r
    fp16 = mybir.dt.float16
    Alu = mybir.AluOpType
    Act = mybir.ActivationFunctionType

    nc = bacc.Bacc(
        get_trn_type() or "TRN2",
        target_bir_lowering=False,
        debug=False,
        enable_asserts=True,
        num_devices=NCORES,
    )

    x_d = nc.dram_tensor("x_rot", [JDOM, F], f32, kind="ExternalInput").ap()
    t_d = nc.dram_tensor("t_mat", [F, BC], f32, kind="ExternalInput").ap()
    s_d = nc.dram_tensor("s_sel", [128, B], fp16, kind="ExternalInput").ap()
    sn_d = nc.dram_tensor("s_neg", [128, B], fp16, kind="ExternalInput").ap()
    sc_d = nc.dram_tensor("s_colw", [128, B], fp16, kind="ExternalInput").ap()
    id_d = nc.dram_tensor("ident", [128, 128], f32, kind="ExternalInput").ap()
    e2_d = nc.dram_tensor("e2_neg", [64, 128], f32, kind="ExternalInput").ap()
    bw_d = nc.dram_tensor("bw_mask", [32, 128], fp16, kind="ExternalInput").ap()
    st_d = nc.dram_tensor("stepb", [32, 512], fp16, kind="ExternalInput").ap()
    or_d = nc.dram_tensor("out_row", [ROWS, B], f32, kind="ExternalOutput").ap()
    oc_d = nc.dram_tensor("out_col", [B, COLW], f32, kind="ExternalOutput").ap()
    u_d = nc.dram_tensor("u_scratch", [2, B, NPAIR], f32).ap()
    u2_d = nc.dram_tensor("u2_scratch", [2, B, NPAIR], f32).ap()

    KCH = F // 128  # 2

    with tile.TileContext(nc) as tc, ExitStack() as ctx:
        singles = ctx.enter_context(tc.tile_pool(name="singles", bufs=1))
        xin = ctx.enter_context(tc.tile_pool(name="xin", bufs=2))
        psum = ctx.enter_context(tc.tile_pool(name="psum", bufs=2, space="PSUM"))
        colp = ctx.enter_context(tc.tile_pool(name="colp", bufs=1, space="PSUM"))
        adp = ctx.enter_context(tc.tile_pool(name="adp", bufs=4))
        escr_p = ctx.enter_context(tc.tile_pool(name="escr", bufs=3))

        # ---- constants -----------------------------------------------------
        s_sel = singles.tile([128, B], fp16)
        nc.sync.dma_start(out=s_sel, in_=s_d)
        s_neg = singles.tile([128, B], fp16)
        nc.sync.dma_start(out=s_neg, in_=sn_d)
        s_colw = singles.tile([128, B], fp16)
        nc.sync.dma_start(out=s_colw, in_=sc_d)
        ident = singles.tile([128, 128], f32)
        nc.sync.dma_start(out=ident, in_=id_d)
        e2_neg = singles.tile([64, 128], f32)
        nc.sync.dma_start(out=e2_neg, in_=e2_d)
        bw_sb = singles.tile([32, 128], fp16)
        nc.sync.dma_start(out=bw_sb, in_=bw_d)
        stepb = singles.tile([32, 512], fp16)
        nc.sync.dma_start(out=stepb, in_=st_d)

        # ---- T (already column-permuted on host) in sbuf: [k, bc] ----------
        t_sb = [singles.tile([128, BC], f32, tag=f"tsb{kc}", name=f"tsb{kc}")
                for kc in range(KCH)]
        t_v = t_d.rearrange("(kc p) q -> kc p q", p=128)
        for kc in range(KCH):
            nc.sync.dma_start(out=t_sb[kc], in_=t_v[kc])

        # ---- MT = (x @ T).T as 4 chunks [128, JDOM]; x transposed on the
        # fly per 512-j block through small rotating buffers
        mt_bf = [singles.tile([128, JDOM], fp16, tag=f"mtb{ch}", name=f"mtb{ch}")
                 for ch in range(NCHUNK)]
        x_v = x_d.rearrange("(t p) f -> t p f", p=128)  # 20 x [128, 256]
        for jt in range(JDOM // 512):
            xTj = [xin.tile([128, 512], f32, tag=f"xTj{kc}", name=f"xTj{kc}")
                   for kc in range(KCH)]
            for t in range(4):
                xt_in = xin.tile([128, F], f32, tag="xtile")
                nc.sync.dma_start(out=xt_in, in_=x_v[jt * 4 + t])
                for kc in range(KCH):
                    pt = psum.tile([128, 1024], f32, tag="ps")
                    nc.tensor.transpose(
                        pt[:, 0:128], xt_in[:, kc * 128:(kc + 1) * 128], ident
                    )
                    nc.scalar.copy(
                        out=xTj[kc][:, t * 128:(t + 1) * 128], in_=pt[:, 0:128]
                    )
            for ch in range(NCHUNK):
                pm = psum.tile([128, 1024], f32, tag="ps")
                for kc in range(KCH):
                    nc.tensor.matmul(
                        pm[:, 0:512],
                        t_sb[kc][:, ch * 128:(ch + 1) * 128],
                        xTj[kc],
                        start=(kc == 0),
                        stop=(kc == KCH - 1),
                    )
                # psum -> fp16 (this rounding defines the kernel's M)
                nc.vector.tensor_copy(
                    out=mt_bf[ch][:, jt * 512:(jt + 1) * 512], in_=pm[:, 0:512]
                )

        # ---- fp32 roundtrips of own-row M values (scalar operands must be
        # f32; equals the fp16 value exactly) + negated copy for relu bias
        mt_f32 = [singles.tile([128, ROWS], f32, tag=f"mtf{ch}", name=f"mtf{ch}")
                  for ch in range(NCHUNK)]
        for ch in range(NCHUNK):
            nc.scalar.copy(out=mt_f32[ch], in_=mt_bf[ch][:, 0:ROWS])
        mtn_f32 = singles.tile([128, ROWS], f32)
        nc.scalar.mul(out=mtn_f32, in_=mt_bf[RELUCH][:, 0:ROWS], mul=-1.0)

        # ---- VT[b, j] = sum_c M[j, b, c] for all local j; f32r for the
        # -V/2 correction matmuls
        vt32 = singles.tile([64, JDOM], f32r)
        e2_r = singles.tile([64, 128], f32r)
        nc.scalar.copy(out=e2_r, in_=e2_neg)
        for jt in range(JDOM // 512):
            pv = psum.tile([128, 1024], f32, tag="ps")
            for ch in range(NCHUNK):
                nc.tensor.matmul(
                    pv[0:64, 0:512],
                    s_sel,
                    mt_bf[ch][:, jt * 512:(jt + 1) * 512],
                    start=(ch == 0),
                    stop=(ch == NCHUNK - 1),
                )
            nc.scalar.copy(
                out=vt32[:, jt * 512:(jt + 1) * 512], in_=pv[0:64, 0:512]
            )

        # ---- u_all[p=(i2,b), pr] = U[2pr+i2, b]  (U = V restricted to own
        # rows); scatter-write so the readback is contiguous
        nc.sync.dma_start(
            out=u_d.rearrange("i2 b pr -> b pr i2"),
            in_=vt32.bitcast(f32)[:, 0:ROWS].rearrange(
                "b (pr i2) -> b pr i2", i2=2),
        )
        u_all = singles.tile([128, NPAIR], f32)
        nc.sync.dma_start(out=u_all, in_=u_d.rearrange("i2 b pr -> (i2 b) pr"))

        # ---- u_mix = U_maxch - U_reluch (bias for the relu window)
        pu = psum.tile([128, 1024], f32, tag="ps")
        for ch in range(NCHUNK):
            nc.tensor.matmul(
                pu[0:64, 0:512],
                s_neg if ch == RELUCH else s_sel,
                mt_bf[ch][:, 0:ROWS],
                start=(ch == 0),
                stop=(ch == NCHUNK - 1),
            )
        umix_src = singles.tile([64, ROWS], f32)
        nc.scalar.copy(out=umix_src, in_=pu[0:64, 0:512])
        nc.sync.dma_start(
            out=u2_d.rearrange("i2 b pr -> b pr i2"),
            in_=umix_src.rearrange("b (pr i2) -> b pr i2", i2=2),
        )
        u_mix = singles.tile([128, NPAIR], f32)
        nc.sync.dma_start(out=u_mix, in_=u2_d.rearrange("i2 b pr -> (i2 b) pr"))

        # ---- per-(i,b) row partial sums: col = pr*3 + window ----------------
        psbuf = singles.tile([128, NPAIR * 3], f32)

        # ---- persistent column-sum accumulators (live across the pr loop)
        colacc = [colp.tile([64, 1024], f32, tag=f"col{w}", name=f"col{w}")
                  for w in range(2)]

        # ---- main loop ------------------------------------------------------
        # windows: A0 = [0,1024) (diag+gap1a, step mask, col sums),
        #          A1 = [1024,2048) (gaps, col sums, relu chunk),
        #          B  = [2048,2560) (gap 4, row sums only)
        for pr in range(NPAIR):
            i0 = 2 * pr
            sk = 256 if i0 >= 256 else 0
            # -- produce pairwise tiles for both rows of the pair ------------
            rhs = {}  # (i2, window) -> list of (tile, joff) rhs sources
            for i2 in range(2):
                i = i0 + i2
                # ch0: TS max over [sk, 2560)
                ad0 = adp.tile([128, JDOM], fp16, tag="ad0")
                nc.vector.tensor_scalar(
                    out=ad0[:, sk:JDOM],
                    in0=mt_bf[0][:, sk:JDOM],
                    scalar1=mt_f32[0][:, i:i + 1],
                    scalar2=None,
                    op0=Alu.max,
                )
                # ch1: STT merge into ad0 -> acc
                acc = adp.tile([128, JDOM], fp16, tag="acc")
                nc.vector.scalar_tensor_tensor(
                    out=acc[:, sk:JDOM],
                    in0=mt_bf[1][:, sk:JDOM],
                    scalar=mt_f32[1][:, i:i + 1],
                    in1=ad0[:, sk:JDOM],
                    op0=Alu.max,
                    op1=Alu.add,
                )
                # ch2: TS max over [sk, 2560)
                ad2 = adp.tile([128, JDOM], fp16, tag="ad2")
                nc.vector.tensor_scalar(
                    out=ad2[:, sk:JDOM],
                    in0=mt_bf[2][:, sk:JDOM],
                    scalar1=mt_f32[2][:, i:i + 1],
                    scalar2=None,
                    op0=Alu.max,
                )
                # ch3: TS max on A0 + B windows; ScalarE relu on A1
                ad3 = adp.tile([128, JDOM], fp16, tag="ad3")
                nc.vector.tensor_scalar(
                    out=ad3[:, sk:1024],
                    in0=mt_bf[3][:, sk:1024],
                    scalar1=mt_f32[3][:, i:i + 1],
                    scalar2=None,
                    op0=Alu.max,
                )
                nc.vector.tensor_scalar(
                    out=ad3[:, 2048:JDOM],
                    in0=mt_bf[3][:, 2048:JDOM],
                    scalar1=mt_f32[3][:, i:i + 1],
                    scalar2=None,
                    op0=Alu.max,
                )
                r3 = escr_p.tile([128, 1024], fp16, tag="r3")
                nc.scalar.activation(
                    out=r3,
                    in_=mt_bf[RELUCH][:, 1024:2048],
                    func=Act.Relu,
                    scale=1.0,
                    bias=mtn_f32[:, i:i + 1],
                )
                rhs[(i2, 0)] = [(acc, 0), (ad2, 0), (ad3, 0)]
                rhs[(i2, 1)] = [(acc, 0), (ad2, 0), (r3, 1024)]
                rhs[(i2, 2)] = [(acc, 0), (ad2, 0), (ad3, 0)]

            # -- windows -----------------------------------------------------
            for w, (j0, j1) in enumerate([(0, 1024), (1024, 2048), (2048, JDOM)]):
                jb0 = j0 + (sk if w == 0 else 0)
                dps = psum.tile([128, 1024], f32, tag="ps")
                # bank-aligned 512-col slices of [jb0, j1)
                js_chunks = []
                s = jb0
                while s < j1:
                    e = min((s // 512 + 1) * 512, j1)
                    js_chunks.append((s, e))
                    s = e
                # main selector matmuls
                for i2 in range(2):
                    for (js0, js1) in js_chunks:
                        for ri, (rt, roff) in enumerate(rhs[(i2, w)]):
                            nc.tensor.matmul(
                                dps[i2 * 64:(i2 + 1) * 64,
                                    js0 - j0:js1 - j0],
                                s_sel,
                                rt[:, js0 - roff:js1 - roff],
                                start=(ri == 0),
                                stop=False,
                                skip_group_check=True,
                            )
                # -V/2 correction
                for (js0, js1) in js_chunks:
                    nc.tensor.matmul(
                        dps[:, js0 - j0:js1 - j0],
                        e2_r,
                        vt32[:, js0:js1],
                        start=False,
                        stop=True,
                        skip_group_check=True,
                    )
                # step mask on A0: +25 for j <= i (covers the self term)
                if w == 0:
                    q = i0 - sk          # 0..254
                    nm = min(-(-(q + 2) // 128) * 128, 512 - sk)
                    off = 256 - q
                    nc.tensor.matmul(
                        dps[:, sk:sk + nm],
                        bw_sb,
                        stepb[:, off:off + nm],
                        start=False,
                        stop=True,
                        skip_group_check=True,
                    )
                # exp(-2P + bias), row sums via accum_out
                escr = escr_p.tile([128, 1024], fp16, tag="escr")
                nc.scalar.activation(
                    out=escr[:, jb0 - j0:j1 - j0],
                    in_=dps[:, jb0 - j0:j1 - j0],
                    func=Act.Exp,
                    scale=-2.0,
                    bias=(u_mix if w == 1 else u_all)[:, pr:pr + 1],
                    accum_out=psbuf[:, pr * 3 + w:pr * 3 + w + 1],
                )
                # column sums (A windows only): accumulate over all pr.
                # NOTE: start=True zeroes the whole 2KB psum bank, so each
                # bank of colacc must see exactly one start (its first MM).
                if w < 2:
                    if w == 0:
                        # bank 0: full [0,512) while escr has it (pr<128),
                        # then only [256,512) (escr [0,256) is stale and the
                        # masked pairs there were finished by pr==127)
                        regions = ([(0, 512)] if pr < 128 else [(256, 512)])
                        regions.append((512, 1024))
                    else:
                        regions = [(0, 512), (512, 1024)]
                    for (c0, c1) in regions:
                        nc.tensor.matmul(
                            colacc[w][:, c0:c1],
                            s_colw,
                            escr[:, c0:c1],
                            start=(pr == 0),
                            stop=(pr == NPAIR - 1),
                            skip_group_check=True,
                        )

        # ---- finish: row part ----------------------------------------------
        red = singles.tile([128, NPAIR], f32)
        tmp = singles.tile([128, NPAIR], f32)
        pv3 = psbuf.rearrange("p (c w) -> p c w", w=3)
        nc.vector.tensor_tensor(
            out=tmp, in0=pv3[:, :, 0], in1=pv3[:, :, 1], op=Alu.add
        )
        nc.vector.tensor_tensor(
            out=red, in0=tmp, in1=pv3[:, :, 2], op=Alu.add
        )
        # red[:, pr]: partition = i2*64 + b. Transpose 128-blocks so the DMA
        # descriptors are contiguous 256B runs.
        o_v = or_d.rearrange("(pr i2) b -> pr i2 b", i2=2)
        for blk in range(NPAIR // 128):
            pt = psum.tile([128, 1024], f32, tag="ps")
            nc.tensor.transpose(
                pt[:, 0:128], red[:, blk * 128:(blk + 1) * 128], ident
            )
            ot = xin.tile([128, 128], f32, tag="otile")
            nc.scalar.copy(out=ot, in_=pt[:, 0:128])
            ot_v = ot.rearrange("q (i2 b) -> q i2 b", i2=2)
            nc.sync.dma_start(out=o_v[blk * 128:(blk + 1) * 128], in_=ot_v)

        # ---- finish: column part -------------------------------------------
        col_sb = singles.tile([64, COLW], f32)
        for w in range(2):
            nc.scalar.copy(
                out=col_sb[:, w * 1024:(w + 1) * 1024], in_=colacc[w]
            )
        nc.sync.dma_start(out=oc_d, in_=col_sb)

    nc.compile()
    return nc


def _get_program():
    if "nc" not in _CACHE:
        _CACHE["nc"] = _build_program()
    return _CACHE["nc"]


def _host_consts():
    s_sel = (np.arange(128)[:, None] // 2 == np.arange(B)[None, :]).astype(
        np.float16
    )
    s_neg = -s_sel
    s_colw = (np.arange(128)[:, None] % 64 == np.arange(B)[None, :]).astype(
        np.float16
    )
    ident = np.eye(128, dtype=np.float32)
    e2_neg = (-0.5 * (np.arange(64)[:, None] == (np.arange(128)[None, :] % 64))
              ).astype(np.float32)
    bw_mask = np.zeros((32, 128), dtype=np.float16)
    bw_mask[0, :64] = 25.0
    bw_mask[1, 64:] = 25.0
    stepb = np.zeros((32, 512), dtype=np.float16)
    stepb[0, : 256 + 1] = 1.0
    stepb[1, : 257 + 1] = 1.0
    return s_sel, s_neg, s_colw, ident, e2_neg, bw_mask, stepb


def _host_inputs(x, T):
    x = np.ascontiguousarray(x, dtype=np.float32)
    # permute T columns: q = ch*128 + b*2 + e  <->  (b, c=2ch+e)
    t_mat = np.ascontiguousarray(
        T.reshape(F, B, NCHUNK, 2).transpose(0, 2, 1, 3).reshape(F, BC),
        dtype=np.float32,
    )
    return x, t_mat


TRACE = bool(int(os.environ.get("KERNEL_TRACE", "0")))
LAST_RESULTS = None


def _make_ntff_hook():
    # the image's antenv lacks axon_hooks, but the injected libaxon_pjrt.so
    # carries the NTFF profile C ABI — drive it via ctypes directly
    import contextlib
    import ctypes

    so_path = "/opt/axon/libaxon_pjrt.so"
    if not os.path.exists(so_path):
        return None
    lib = ctypes.CDLL(so_path)
    if not hasattr(lib, "axon_start_nrt_profile"):
        return None
    lib.axon_start_nrt_profile.argtypes = [
        ctypes.POINTER(ctypes.c_int64),
        ctypes.c_size_t,
    ]
    lib.axon_start_nrt_profile.restype = ctypes.c_int64
    lib.axon_stop_nrt_profile.argtypes = [ctypes.c_char_p]
    lib.axon_stop_nrt_profile.restype = ctypes.c_int64

    @contextlib.contextmanager
    def _hook(output_dir, device_ids):
        import jax

        jax.devices()
        if device_ids:
            ids = (ctypes.c_int64 * len(device_ids))(*device_ids)
            rc = lib.axon_start_nrt_profile(ids, len(device_ids))
        else:
            rc = lib.axon_start_nrt_profile(None, 0)
        if rc != 0:
            raise RuntimeError(f"axon_start_nrt_profile rc={rc}")
        try:
            yield
        finally:
            n = lib.axon_stop_nrt_profile(str(output_dir).encode())
            print(f"profile: {n} file(s) written to {output_dir}")

    return _hook


def _ensure_axon_hook_stub():
    import importlib
    import sys
    import types

    try:
        importlib.import_module("antenv.axon_hooks")
    except ModuleNotFoundError:
        stub = types.ModuleType("antenv.axon_hooks")
        stub.get_axon_ntff_profile_hook = _make_ntff_hook
        sys.modules["antenv.axon_hooks"] = stub


def kernel(x: np.ndarray, T: np.ndarray) -> np.ndarray:
    global LAST_RESULTS
    _ensure_axon_hook_stub()
    from concourse.bass_utils import run_bass_kernel_spmd

    nc = _get_program()
    x, t_mat = _host_inputs(x, T)
    s_sel, s_neg, s_colw, ident, e2_neg, bw_mask, stepb = _host_consts()

    in_maps = []
    for k in range(NCORES):
        x_rot = np.roll(x, -ROWS * k, axis=0)[:JDOM] if k else x[:JDOM]
        in_maps.append(
            {"x_rot": np.ascontiguousarray(x_rot), "t_mat": t_mat,
             "s_sel": s_sel, "s_neg": s_neg, "s_colw": s_colw,
             "ident": ident, "e2_neg": e2_neg, "bw_mask": bw_mask,
             "stepb": stepb}
        )

    res = run_bass_kernel_spmd(
        nc, in_maps, core_ids=list(range(NCORES)), trace=TRACE
    )
    LAST_RESULTS = res
    out = np.zeros((N, B), dtype=np.float64)
    for k in range(NCORES):
        out[k * ROWS:(k + 1) * ROWS] += res.results[k]["out_row"].astype(
            np.float64
        )
        colp = res.results[k]["out_col"].astype(np.float64).T  # [COLW, B]
        gidx = (k * ROWS + np.arange(COLW)) % N
        np.add.at(out, gidx, colp)
    return np.concatenate([x, out.astype(np.float32)], axis=1)


# revision 17
# speedup vs baseline: 1.5322x; 1.1052x over previous
"""Minibatch-discrimination kernel for Trainium2 (8 NeuronCores, SPMD), v2.

Math: M = einsum('nf,fbc->nbc', x, T); d[i,j,b] = sum_c |M[i,b,c]-M[j,b,c]|;
out[i,b] = sum_j exp(-d[i,j,b]) - 1; return concat([x, out], axis=1).

v2 exploits d(i,j)=d(j,i): core k only computes pairs against local j in
[0, 2560) (own block, gaps 1-3, gap 4) instead of [0, 4096):
  - j in [0, 512): diagonal block, upper triangle only (a step-mask matmul
    adds +25 to the psum for j<=i, killing exp); row sums cover j>i, column
    sums cover i<j. The self term is never computed, so no -1 at the end.
  - j in [512, 2048): gaps 1-3, full; row sums for own rows + column sums
    emitted as partial outputs for blocks k+1..k+3 (host adds them).
  - j in [2048, 2560): gap 4, row sums only (both endpoint cores compute
    their own rows against the partner block).
Host combines row parts + rotated column parts.

The pairwise pass is split across three engines (tunable per-window):
  - DVE tensor_scalar max (4x fp16): ad = max(mt_j, mt_i)
  - DVE scalar_tensor_tensor: acc = max(mt_j, mt_i) + ad_prev (merges two
    chunks into one matmul rhs, 2x mode)
  - ScalarE relu: R = relu(mt_j - mt_i); |a| = a + 2*relu(-a) flips that
    chunk's U_i sign in the exp bias: bias = U_maxch - U_reluch.
PE sums partition pairs via a 0/1 selector (K=128 -> M=64) into PSUM
([128=(i2,64b), jw] tiles), adds -V_j/2 via a K=64 matmul, ScalarE computes
exp(-2P + bias) with the j-sum via accum_out, and column sums accumulate
over all pr in persistent PSUM tiles via a ones-pair selector matmul over
the fp16 exp tile.
"""

import os
from contextlib import ExitStack

import numpy as np

N, F, B, C = 4096, 256, 64, 8
NCORES = 8
ROWS = N // NCORES          # 512 output rows per core
JDOM = ROWS * 5             # 2560: local j domain (diag + gaps 1-3 + gap 4)
COLW = ROWS * 4             # 2048: j range with column-sum partials
BC = B * C                  # 512
NCHUNK = BC // 128          # 4 partition-chunks of M.T
NPAIR = ROWS // 2           # 256 (two i's fill one 128-partition psum tile)
RELUCH = 3                  # chunk computed by ScalarE relu on window A1

_CACHE = {}


def _build_program():
    import concourse.bacc as bacc
    import concourse.tile as tile
    from concourse import mybir
    from concourse._compat import get_trn_type

    f32 = mybir.dt.float32
    f32r = mybir.dt.float32r
    fp16 = mybir.dt.float16
    Alu = mybir.AluOpType
    Act = mybir.ActivationFunctionType

    nc = bacc.Bacc(
        get_trn_type() or "TRN2",
        target_bir_lowering=False,
        debug=False,
        enable_asserts=True,
        num_devices=NCORES,
    )

    x_d = nc.dram_tensor("x_rot", [JDOM, F], f32, kind="ExternalInput").ap()
    t_d = nc.dram_tensor("t_mat", [F, BC], f32, kind="ExternalInput").ap()
    s_d = nc.dram_tensor("s_sel", [128, B], fp16, kind="ExternalInput").ap()
    sn_d = nc.dram_tensor("s_neg", [128, B], fp16, kind="ExternalInput").ap()
    sc_d = nc.dram_tensor("s_colw", [128, B], fp16, kind="ExternalInput").ap()
    id_d = nc.dram_tensor("ident", [128, 128], f32, kind="ExternalInput").ap()
    e2_d = nc.dram_tensor("e2_neg", [64, 128], f32, kind="ExternalInput").ap()
    bw_d = nc.dram_tensor("bw_mask", [32, 128], fp16, kind="ExternalInput").ap()
    st_d = nc.dram_tensor("stepb", [32, 512], fp16, kind="ExternalInput").ap()
    or_d = nc.dram_tensor("out_row", [ROWS, B], f32, kind="ExternalOutput").ap()
    oc_d = nc.dram_tensor("out_col", [B, COLW], f32, kind="ExternalOutput").ap()
    u_d = nc.dram_tensor("u_scratch", [2, B, NPAIR], f32).ap()
    u2_d = nc.dram_tensor("u2_scratch", [2, B, NPAIR], f32).ap()

    KCH = F // 128  # 2

    with tile.TileContext(nc) as tc, ExitStack() as ctx:
        singles = ctx.enter_context(tc.tile_pool(name="singles", bufs=1))
        xin = ctx.enter_context(tc.tile_pool(name="xin", bufs=2))
        psum = ctx.enter_context(tc.tile_pool(name="psum", bufs=3, space="PSUM"))
        colp = ctx.enter_context(tc.tile_pool(name="colp", bufs=1, space="PSUM"))
        adp_s = ctx.enter_context(tc.tile_pool(name="adp_s", bufs=2))
        adp = ctx.enter_context(tc.tile_pool(name="adp", bufs=3))
        escr_p = ctx.enter_context(tc.tile_pool(name="escr", bufs=3))

        # ---- constants -----------------------------------------------------
        s_sel = singles.tile([128, B], fp16)
        nc.sync.dma_start(out=s_sel, in_=s_d)
        s_neg = singles.tile([128, B], fp16)
        nc.sync.dma_start(out=s_neg, in_=sn_d)
        s_colw = singles.tile([128, B], fp16)
        nc.sync.dma_start(out=s_colw, in_=sc_d)
        ident = singles.tile([128, 128], f32)
        nc.sync.dma_start(out=ident, in_=id_d)
        e2_neg = singles.tile([64, 128], f32)
        nc.sync.dma_start(out=e2_neg, in_=e2_d)
        bw_sb = singles.tile([32, 128], fp16)
        nc.sync.dma_start(out=bw_sb, in_=bw_d)
        stepb = singles.tile([32, 512], fp16)
        nc.sync.dma_start(out=stepb, in_=st_d)

        # ---- T (already column-permuted on host) in sbuf: [k, bc] ----------
        t_sb = [singles.tile([128, BC], f32, tag=f"tsb{kc}", name=f"tsb{kc}")
                for kc in range(KCH)]
        t_v = t_d.rearrange("(kc p) q -> kc p q", p=128)
        for kc in range(KCH):
            nc.sync.dma_start(out=t_sb[kc], in_=t_v[kc])

        # ---- MT = (x @ T).T as 4 chunks [128, JDOM]; x transposed on the
        # fly per 512-j block through small rotating buffers
        mt_bf = [singles.tile([128, JDOM], fp16, tag=f"mtb{ch}", name=f"mtb{ch}")
                 for ch in range(NCHUNK)]
        x_v = x_d.rearrange("(t p) f -> t p f", p=128)  # 20 x [128, 256]
        for jt in range(JDOM // 512):
            xTj = [xin.tile([128, 512], f32, tag=f"xTj{kc}", name=f"xTj{kc}")
                   for kc in range(KCH)]
            for t in range(4):
                xt_in = xin.tile([128, F], f32, tag="xtile")
                nc.sync.dma_start(out=xt_in, in_=x_v[jt * 4 + t])
                for kc in range(KCH):
                    pt = psum.tile([128, 1024], f32, tag="ps")
                    nc.tensor.transpose(
                        pt[:, 0:128], xt_in[:, kc * 128:(kc + 1) * 128], ident
                    )
                    nc.scalar.copy(
                        out=xTj[kc][:, t * 128:(t + 1) * 128], in_=pt[:, 0:128]
                    )
            for ch in range(NCHUNK):
                pm = psum.tile([128, 1024], f32, tag="ps")
                for kc in range(KCH):
                    nc.tensor.matmul(
                        pm[:, 0:512],
                        t_sb[kc][:, ch * 128:(ch + 1) * 128],
                        xTj[kc],
                        start=(kc == 0),
                        stop=(kc == KCH - 1),
                    )
                # psum -> fp16 (this rounding defines the kernel's M)
                nc.vector.tensor_copy(
                    out=mt_bf[ch][:, jt * 512:(jt + 1) * 512], in_=pm[:, 0:512]
                )

        # ---- fp32 roundtrips of own-row M values (scalar operands must be
        # f32; equals the fp16 value exactly) + negated copy for relu bias
        mt_f32 = [singles.tile([128, ROWS], f32, tag=f"mtf{ch}", name=f"mtf{ch}")
                  for ch in range(NCHUNK)]
        for ch in range(NCHUNK):
            nc.scalar.copy(out=mt_f32[ch], in_=mt_bf[ch][:, 0:ROWS])
        mtn_f32 = singles.tile([128, ROWS], f32)
        nc.scalar.mul(out=mtn_f32, in_=mt_bf[RELUCH][:, 0:ROWS], mul=-1.0)

        # ---- VT[b, j] = sum_c M[j, b, c] for all local j; f32r for the
        # -V/2 correction matmuls
        vt32 = singles.tile([64, JDOM], f32r)
        e2_r = singles.tile([64, 128], f32r)
        nc.scalar.copy(out=e2_r, in_=e2_neg)
        for jt in range(JDOM // 512):
            pv = psum.tile([128, 1024], f32, tag="ps")
            for ch in range(NCHUNK):
                nc.tensor.matmul(
                    pv[0:64, 0:512],
                    s_sel,
                    mt_bf[ch][:, jt * 512:(jt + 1) * 512],
                    start=(ch == 0),
                    stop=(ch == NCHUNK - 1),
                )
            nc.scalar.copy(
                out=vt32[:, jt * 512:(jt + 1) * 512], in_=pv[0:64, 0:512]
            )

        # ---- u_all[p=(i2,b), pr] = U[2pr+i2, b]  (U = V restricted to own
        # rows); scatter-write so the readback is contiguous
        nc.sync.dma_start(
            out=u_d.rearrange("i2 b pr -> b pr i2"),
            in_=vt32.bitcast(f32)[:, 0:ROWS].rearrange(
                "b (pr i2) -> b pr i2", i2=2),
        )
        u_all = singles.tile([128, NPAIR], f32)
        nc.sync.dma_start(out=u_all, in_=u_d.rearrange("i2 b pr -> (i2 b) pr"))

        # ---- u_mix = U_maxch - U_reluch (bias for the relu window)
        pu = psum.tile([128, 1024], f32, tag="ps")
        for ch in range(NCHUNK):
            nc.tensor.matmul(
                pu[0:64, 0:512],
                s_neg if ch == RELUCH else s_sel,
                mt_bf[ch][:, 0:ROWS],
                start=(ch == 0),
                stop=(ch == NCHUNK - 1),
            )
        umix_src = singles.tile([64, ROWS], f32)
        nc.scalar.copy(out=umix_src, in_=pu[0:64, 0:512])
        nc.sync.dma_start(
            out=u2_d.rearrange("i2 b pr -> b pr i2"),
            in_=umix_src.rearrange("b (pr i2) -> b pr i2", i2=2),
        )
        u_mix = singles.tile([128, NPAIR], f32)
        nc.sync.dma_start(out=u_mix, in_=u2_d.rearrange("i2 b pr -> (i2 b) pr"))

        # ---- per-(i,b) row partial sums: col = pr*3 + window ----------------
        psbuf = singles.tile([128, NPAIR * 3], f32)

        # ---- persistent column-sum accumulators (live across the pr loop);
        # both packed into one [128, 1024] psum tile: window A0 sums on
        # partitions 0-63, window A1 on 64-127 (matmul tile_position derives
        # from out.base_partition)
        colt = colp.tile([128, 1024], f32)
        colacc = [colt[0:64, :], colt[64:128, :]]

        # ---- main loop ------------------------------------------------------
        # windows: A0 = [0,1024) (diag+gap1a, step mask, col sums),
        #          A1 = [1024,2048) (gaps, col sums, relu chunk),
        #          B  = [2048,2560) (gap 4, row sums only)
        for pr in range(NPAIR):
            i0 = 2 * pr
            sk = min((i0 // 128) * 128, 384)
            # -- produce pairwise tiles for both rows of the pair ------------
            rhs = {}  # (i2, window) -> list of (tile, joff) rhs sources
            for i2 in range(2):
                i = i0 + i2
                # ch0/ch1: TS max over [sk, 2560), merged by a TT add (TT is
                # 2x fp16 mode; scalar_tensor_tensor measured 1x -> avoided)
                ad0 = adp_s.tile([128, JDOM], fp16, tag="ad0")
                nc.vector.tensor_scalar(
                    out=ad0[:, sk:JDOM],
                    in0=mt_bf[0][:, sk:JDOM],
                    scalar1=mt_f32[0][:, i:i + 1],
                    scalar2=None,
                    op0=Alu.max,
                )
                ad1 = adp_s.tile([128, JDOM], fp16, tag="ad1")
                nc.vector.tensor_scalar(
                    out=ad1[:, sk:JDOM],
                    in0=mt_bf[1][:, sk:JDOM],
                    scalar1=mt_f32[1][:, i:i + 1],
                    scalar2=None,
                    op0=Alu.max,
                )
                m01 = adp.tile([128, JDOM], fp16, tag="m01")
                nc.vector.tensor_tensor(
                    out=m01[:, sk:JDOM],
                    in0=ad0[:, sk:JDOM],
                    in1=ad1[:, sk:JDOM],
                    op=Alu.add,
                )
                # ch2: TS max over [sk, 2560)
                ad2 = adp.tile([128, JDOM], fp16, tag="ad2")
                nc.vector.tensor_scalar(
                    out=ad2[:, sk:JDOM],
                    in0=mt_bf[2][:, sk:JDOM],
                    scalar1=mt_f32[2][:, i:i + 1],
                    scalar2=None,
                    op0=Alu.max,
                )
                # ch3: TS max on A0 + B windows; ScalarE relu on A1
                ad3 = adp.tile([128, JDOM], fp16, tag="ad3")
                nc.vector.tensor_scalar(
                    out=ad3[:, sk:1024],
                    in0=mt_bf[3][:, sk:1024],
                    scalar1=mt_f32[3][:, i:i + 1],
                    scalar2=None,
                    op0=Alu.max,
                )
                nc.vector.tensor_scalar(
                    out=ad3[:, 2048:JDOM],
                    in0=mt_bf[3][:, 2048:JDOM],
                    scalar1=mt_f32[3][:, i:i + 1],
                    scalar2=None,
                    op0=Alu.max,
                )
                r3 = escr_p.tile([128, 1024], fp16, tag="r3")
                nc.scalar.activation(
                    out=r3,
                    in_=mt_bf[RELUCH][:, 1024:2048],
                    func=Act.Relu,
                    scale=1.0,
                    bias=mtn_f32[:, i:i + 1],
                )
                rhs[(i2, 0)] = [(m01, 0), (ad2, 0), (ad3, 0)]
                rhs[(i2, 1)] = [(m01, 0), (ad2, 0), (r3, 1024)]
                rhs[(i2, 2)] = [(m01, 0), (ad2, 0), (ad3, 0)]

            # -- windows -----------------------------------------------------
            for w, (j0, j1) in enumerate([(0, 1024), (1024, 2048), (2048, JDOM)]):
                jb0 = j0 + (sk if w == 0 else 0)
                dps = psum.tile([128, 1024], f32, tag="ps")
                # bank-aligned 512-col slices of [jb0, j1)
                js_chunks = []
                s = jb0
                while s < j1:
                    e = min((s // 512 + 1) * 512, j1)
                    js_chunks.append((s, e))
                    s = e
                # main selector matmuls
                for i2 in range(2):
                    for (js0, js1) in js_chunks:
                        for ri, (rt, roff) in enumerate(rhs[(i2, w)]):
                            nc.tensor.matmul(
                                dps[i2 * 64:(i2 + 1) * 64,
                                    js0 - j0:js1 - j0],
                                s_sel,
                                rt[:, js0 - roff:js1 - roff],
                                start=(ri == 0),
                                stop=False,
                                skip_group_check=True,
                            )
                # -V/2 correction
                for (js0, js1) in js_chunks:
                    nc.tensor.matmul(
                        dps[:, js0 - j0:js1 - j0],
                        e2_r,
                        vt32[:, js0:js1],
                        start=False,
                        stop=True,
                        skip_group_check=True,
                    )
                # step mask on A0: +25 for j <= i (covers the self term)
                if w == 0:
                    q = i0 - sk          # 0..126
                    nm = 128
                    off = 128 - q
                    nc.tensor.matmul(
                        dps[:, sk:sk + nm],
                        bw_sb,
                        stepb[:, off:off + nm],
                        start=False,
                        stop=True,
                        skip_group_check=True,
                    )
                # exp(-2P + bias), row sums via accum_out
                escr = escr_p.tile([128, 1024], fp16, tag="escr")
                nc.scalar.activation(
                    out=escr[:, jb0 - j0:j1 - j0],
                    in_=dps[:, jb0 - j0:j1 - j0],
                    func=Act.Exp,
                    scale=-2.0,
                    bias=(u_mix if w == 1 else u_all)[:, pr:pr + 1],
                    accum_out=psbuf[:, pr * 3 + w:pr * 3 + w + 1],
                )
                # column sums (A windows only): accumulate over all pr.
                # NOTE: start=True zeroes the whole 2KB psum bank, so each
                # bank of colacc must see exactly one start (its first MM).
                if w < 2:
                    if w == 0:
                        # bank 0: only [sk, 512) is valid escr this pr; cols
                        # below sk were completed by earlier prs
                        regions = [(sk, 512), (512, 1024)]
                    else:
                        regions = [(0, 512), (512, 1024)]
                    for (c0, c1) in regions:
                        nc.tensor.matmul(
                            colacc[w][:, c0:c1],
                            s_colw,
                            escr[:, c0:c1],
                            start=(pr == 0),
                            stop=(pr == NPAIR - 1),
                            skip_group_check=True,
                        )

        # ---- finish: row part ----------------------------------------------
        red = singles.tile([128, NPAIR], f32)
        tmp = singles.tile([128, NPAIR], f32)
        pv3 = psbuf.rearrange("p (c w) -> p c w", w=3)
        nc.vector.tensor_tensor(
            out=tmp, in0=pv3[:, :, 0], in1=pv3[:, :, 1], op=Alu.add
        )
        nc.vector.tensor_tensor(
            out=red, in0=tmp, in1=pv3[:, :, 2], op=Alu.add
        )
        # red[:, pr]: partition = i2*64 + b. Transpose 128-blocks so the DMA
        # descriptors are contiguous 256B runs.
        o_v = or_d.rearrange("(pr i2) b -> pr i2 b", i2=2)
        for blk in range(NPAIR // 128):
            pt = psum.tile([128, 1024], f32, tag="ps")
            nc.tensor.transpose(
                pt[:, 0:128], red[:, blk * 128:(blk + 1) * 128], ident
            )
            ot = xin.tile([128, 128], f32, tag="otile")
            nc.scalar.copy(out=ot, in_=pt[:, 0:128])
            ot_v = ot.rearrange("q (i2 b) -> q i2 b", i2=2)
            nc.sync.dma_start(out=o_v[blk * 128:(blk + 1) * 128], in_=ot_v)

        # ---- finish: column part (partition-aligned copy, remap in the DMA:
        # partitions (w b), free j -> out_col[b, w*1024 + j])
        col_sb = singles.tile([128, 1024], f32)
        nc.scalar.copy(out=col_sb, in_=colt)
        nc.sync.dma_start(out=oc_d[:, 0:1024], in_=col_sb[0:64, :])
        nc.sync.dma_start(out=oc_d[:, 1024:2048], in_=col_sb[64:128, :])

    nc.compile()
    return nc


def _get_program():
    if "nc" not in _CACHE:
        _CACHE["nc"] = _build_program()
    return _CACHE["nc"]


def _host_consts():
    s_sel = (np.arange(128)[:, None] // 2 == np.arange(B)[None, :]).astype(
        np.float16
    )
    s_neg = -s_sel
    s_colw = (np.arange(128)[:, None] % 64 == np.arange(B)[None, :]).astype(
        np.float16
    )
    ident = np.eye(128, dtype=np.float32)
    e2_neg = (-0.5 * (np.arange(64)[:, None] == (np.arange(128)[None, :] % 64))
              ).astype(np.float32)
    bw_mask = np.zeros((32, 128), dtype=np.float16)
    bw_mask[0, :64] = 25.0
    bw_mask[1, 64:] = 25.0
    stepb = np.zeros((32, 512), dtype=np.float16)
    stepb[0, : 128 + 1] = 1.0
    stepb[1, : 129 + 1] = 1.0
    return s_sel, s_neg, s_colw, ident, e2_neg, bw_mask, stepb


def _host_inputs(x, T):
    x = np.ascontiguousarray(x, dtype=np.float32)
    # permute T columns: q = ch*128 + b*2 + e  <->  (b, c=2ch+e)
    t_mat = np.ascontiguousarray(
        T.reshape(F, B, NCHUNK, 2).transpose(0, 2, 1, 3).reshape(F, BC),
        dtype=np.float32,
    )
    return x, t_mat


TRACE = bool(int(os.environ.get("KERNEL_TRACE", "0")))
LAST_RESULTS = None


def _make_ntff_hook():
    # the image's antenv lacks axon_hooks, but the injected libaxon_pjrt.so
    # carries the NTFF profile C ABI — drive it via ctypes directly
    import contextlib
    import ctypes

    so_path = "/opt/axon/libaxon_pjrt.so"
    if not os.path.exists(so_path):
        return None
    lib = ctypes.CDLL(so_path)
    if not hasattr(lib, "axon_start_nrt_profile"):
        return None
    lib.axon_start_nrt_profile.argtypes = [
        ctypes.POINTER(ctypes.c_int64),
        ctypes.c_size_t,
    ]
    lib.axon_start_nrt_profile.restype = ctypes.c_int64
    lib.axon_stop_nrt_profile.argtypes = [ctypes.c_char_p]
    lib.axon_stop_nrt_profile.restype = ctypes.c_int64

    @contextlib.contextmanager
    def _hook(output_dir, device_ids):
        import jax

        jax.devices()
        if device_ids:
            ids = (ctypes.c_int64 * len(device_ids))(*device_ids)
            rc = lib.axon_start_nrt_profile(ids, len(device_ids))
        else:
            rc = lib.axon_start_nrt_profile(None, 0)
        if rc != 0:
            raise RuntimeError(f"axon_start_nrt_profile rc={rc}")
        try:
            yield
        finally:
            n = lib.axon_stop_nrt_profile(str(output_dir).encode())
            print(f"profile: {n} file(s) written to {output_dir}")

    return _hook


def _ensure_axon_hook_stub():
    import importlib
    import sys
    import types

    try:
        importlib.import_module("antenv.axon_hooks")
    except ModuleNotFoundError:
        stub = types.ModuleType("antenv.axon_hooks")
        stub.get_axon_ntff_profile_hook = _make_ntff_hook
        sys.modules["antenv.axon_hooks"] = stub


def kernel(x: np.ndarray, T: np.ndarray) -> np.ndarray:
    global LAST_RESULTS
    _ensure_axon_hook_stub()
    from concourse.bass_utils import run_bass_kernel_spmd

    nc = _get_program()
    x, t_mat = _host_inputs(x, T)
    s_sel, s_neg, s_colw, ident, e2_neg, bw_mask, stepb = _host_consts()

    in_maps = []
    for k in range(NCORES):
        x_rot = np.roll(x, -ROWS * k, axis=0)[:JDOM] if k else x[:JDOM]
        in_maps.append(
            {"x_rot": np.ascontiguousarray(x_rot), "t_mat": t_mat,
             "s_sel": s_sel, "s_neg": s_neg, "s_colw": s_colw,
             "ident": ident, "e2_neg": e2_neg, "bw_mask": bw_mask,
             "stepb": stepb}
        )

    res = run_bass_kernel_spmd(
        nc, in_maps, core_ids=list(range(NCORES)), trace=TRACE
    )
    LAST_RESULTS = res
    out = np.zeros((N, B), dtype=np.float64)
    for k in range(NCORES):
        out[k * ROWS:(k + 1) * ROWS] += res.results[k]["out_row"].astype(
            np.float64
        )
        colp = res.results[k]["out_col"].astype(np.float64).T  # [COLW, B]
        gidx = (k * ROWS + np.arange(COLW)) % N
        np.add.at(out, gidx, colp)
    return np.concatenate([x, out.astype(np.float32)], axis=1)


# revision 19
# speedup vs baseline: 1.8539x; 1.2100x over previous
"""Minibatch-discrimination kernel for Trainium2 (8 NeuronCores, SPMD), v2.

Math: M = einsum('nf,fbc->nbc', x, T); d[i,j,b] = sum_c |M[i,b,c]-M[j,b,c]|;
out[i,b] = sum_j exp(-d[i,j,b]) - 1; return concat([x, out], axis=1).

v2 exploits d(i,j)=d(j,i): core k only computes pairs against local j in
[0, 2560) (own block, gaps 1-3, gap 4) instead of [0, 4096):
  - j in [0, 512): diagonal block, upper triangle only (a step-mask matmul
    adds +25 to the psum for j<=i, killing exp); row sums cover j>i, column
    sums cover i<j. The self term is never computed, so no -1 at the end.
  - j in [512, 2048): gaps 1-3, full; row sums for own rows + column sums
    emitted as partial outputs for blocks k+1..k+3 (host adds them).
  - j in [2048, 2560): gap 4, row sums only (both endpoint cores compute
    their own rows against the partner block).
Host combines row parts + rotated column parts.

The pairwise pass is split across three engines (tunable per-window):
  - DVE tensor_scalar max (4x fp16): ad = max(mt_j, mt_i)
  - DVE scalar_tensor_tensor: acc = max(mt_j, mt_i) + ad_prev (merges two
    chunks into one matmul rhs, 2x mode)
  - ScalarE relu: R = relu(mt_j - mt_i); |a| = a + 2*relu(-a) flips that
    chunk's U_i sign in the exp bias: bias = U_maxch - U_reluch.
PE sums partition pairs via a 0/1 selector (K=128 -> M=64) into PSUM
([128=(i2,64b), jw] tiles), adds -V_j/2 via a K=64 matmul, ScalarE computes
exp(-2P + bias) with the j-sum via accum_out, and column sums accumulate
over all pr in persistent PSUM tiles via a ones-pair selector matmul over
the fp16 exp tile.
"""

import os
from contextlib import ExitStack

import numpy as np

N, F, B, C = 4096, 256, 64, 8
NCORES = 8
ROWS = N // NCORES          # 512 output rows per core
JDOM = ROWS * 5             # 2560: local j domain (diag + gaps 1-3 + gap 4)
COLW = ROWS * 4             # 2048: j range with column-sum partials
BC = B * C                  # 512
NCHUNK = BC // 128          # 4 partition-chunks of M.T
NPAIR = ROWS // 2           # 256 (two i's fill one 128-partition psum tile)
RELUCH = 3                  # chunk computed by ScalarE relu on window A1

_CACHE = {}


def _build_program():
    import concourse.bacc as bacc
    import concourse.tile as tile
    from concourse import mybir
    from concourse._compat import get_trn_type

    f32 = mybir.dt.float32
    f32r = mybir.dt.float32r
    fp16 = mybir.dt.float16
    Alu = mybir.AluOpType
    Act = mybir.ActivationFunctionType

    nc = bacc.Bacc(
        get_trn_type() or "TRN2",
        target_bir_lowering=False,
        debug=False,
        enable_asserts=True,
        num_devices=NCORES,
    )

    x_d = nc.dram_tensor("x_rot", [JDOM, F], f32, kind="ExternalInput").ap()
    t_d = nc.dram_tensor("t_mat", [F, BC], f32, kind="ExternalInput").ap()
    s_d = nc.dram_tensor("s_sel", [128, B], fp16, kind="ExternalInput").ap()
    sn_d = nc.dram_tensor("s_neg", [128, B], fp16, kind="ExternalInput").ap()
    sc_d = nc.dram_tensor("s_colw", [128, B], fp16, kind="ExternalInput").ap()
    id_d = nc.dram_tensor("ident", [128, 128], f32, kind="ExternalInput").ap()
    e2_d = nc.dram_tensor("e2_neg", [64, 128], f32, kind="ExternalInput").ap()
    bw_d = nc.dram_tensor("bw_mask", [32, 128], fp16, kind="ExternalInput").ap()
    st_d = nc.dram_tensor("stepb", [32, 512], fp16, kind="ExternalInput").ap()
    or_d = nc.dram_tensor("out_row", [ROWS, B], f32, kind="ExternalOutput").ap()
    oc_d = nc.dram_tensor("out_col", [B, COLW], f32, kind="ExternalOutput").ap()

    KCH = F // 128  # 2

    with tile.TileContext(nc) as tc, ExitStack() as ctx:
        singles = ctx.enter_context(tc.tile_pool(name="singles", bufs=1))
        xin = ctx.enter_context(tc.tile_pool(name="xin", bufs=2))
        psum = ctx.enter_context(tc.tile_pool(name="psum", bufs=3, space="PSUM"))
        colp = ctx.enter_context(tc.tile_pool(name="colp", bufs=1, space="PSUM"))
        adp_s = ctx.enter_context(tc.tile_pool(name="adp_s", bufs=2))
        adp = ctx.enter_context(tc.tile_pool(name="adp", bufs=3))
        escr_p = ctx.enter_context(tc.tile_pool(name="escr", bufs=3))

        # ---- constants -----------------------------------------------------
        s_sel = singles.tile([128, B], fp16)
        nc.sync.dma_start(out=s_sel, in_=s_d)
        s_neg = singles.tile([128, B], fp16)
        nc.sync.dma_start(out=s_neg, in_=sn_d)
        s_colw = singles.tile([128, B], fp16)
        nc.sync.dma_start(out=s_colw, in_=sc_d)
        ident = singles.tile([128, 128], f32)
        nc.sync.dma_start(out=ident, in_=id_d)
        e2_neg = singles.tile([64, 128], f32)
        nc.sync.dma_start(out=e2_neg, in_=e2_d)
        bw_sb = singles.tile([32, 128], fp16)
        nc.sync.dma_start(out=bw_sb, in_=bw_d)
        stepb = singles.tile([32, 512], fp16)
        nc.sync.dma_start(out=stepb, in_=st_d)

        # ---- T (already column-permuted on host) in sbuf: [k, bc] ----------
        t_sb = [singles.tile([128, BC], f32, tag=f"tsb{kc}", name=f"tsb{kc}")
                for kc in range(KCH)]
        t_v = t_d.rearrange("(kc p) q -> kc p q", p=128)
        for kc in range(KCH):
            nc.sync.dma_start(out=t_sb[kc], in_=t_v[kc])

        # ---- MT = (x @ T).T as 4 chunks [128, JDOM]; x transposed on the
        # fly per 512-j block through small rotating buffers
        mt_bf = [singles.tile([128, JDOM], fp16, tag=f"mtb{ch}", name=f"mtb{ch}")
                 for ch in range(NCHUNK)]
        x_v = x_d.rearrange("(t p) f -> t p f", p=128)  # 20 x [128, 256]
        for jt in range(JDOM // 512):
            xTj = [xin.tile([128, 512], f32, tag=f"xTj{kc}", name=f"xTj{kc}")
                   for kc in range(KCH)]
            for t in range(4):
                xt_in = xin.tile([128, F], f32, tag="xtile")
                nc.sync.dma_start(out=xt_in, in_=x_v[jt * 4 + t])
                for kc in range(KCH):
                    pt = psum.tile([128, 1024], f32, tag="ps")
                    nc.tensor.transpose(
                        pt[:, 0:128], xt_in[:, kc * 128:(kc + 1) * 128], ident
                    )
                    nc.scalar.copy(
                        out=xTj[kc][:, t * 128:(t + 1) * 128], in_=pt[:, 0:128]
                    )
            for ch in range(NCHUNK):
                pm = psum.tile([128, 1024], f32, tag="ps")
                for kc in range(KCH):
                    nc.tensor.matmul(
                        pm[:, 0:512],
                        t_sb[kc][:, ch * 128:(ch + 1) * 128],
                        xTj[kc],
                        start=(kc == 0),
                        stop=(kc == KCH - 1),
                    )
                # psum -> fp16 (this rounding defines the kernel's M)
                nc.vector.tensor_copy(
                    out=mt_bf[ch][:, jt * 512:(jt + 1) * 512], in_=pm[:, 0:512]
                )

        # ---- fp32 roundtrips of own-row M values (scalar operands must be
        # f32; equals the fp16 value exactly) + negated copy for relu bias
        mt_f32 = [singles.tile([128, ROWS], f32, tag=f"mtf{ch}", name=f"mtf{ch}")
                  for ch in range(NCHUNK)]
        for ch in range(NCHUNK):
            nc.scalar.copy(out=mt_f32[ch], in_=mt_bf[ch][:, 0:ROWS])
        mtn_f32 = singles.tile([128, ROWS], f32)
        nc.scalar.mul(out=mtn_f32, in_=mt_bf[RELUCH][:, 0:ROWS], mul=-1.0)

        # ---- VT[b, j] = sum_c M[j, b, c] for all local j; f32r for the
        # -V/2 correction matmuls
        vt32 = singles.tile([64, JDOM], f32r)
        e2_r = singles.tile([64, 128], f32r)
        nc.scalar.copy(out=e2_r, in_=e2_neg)
        for jt in range(JDOM // 512):
            pv = psum.tile([128, 1024], f32, tag="ps")
            for ch in range(NCHUNK):
                nc.tensor.matmul(
                    pv[0:64, 0:512],
                    s_sel,
                    mt_bf[ch][:, jt * 512:(jt + 1) * 512],
                    start=(ch == 0),
                    stop=(ch == NCHUNK - 1),
                )
            nc.scalar.copy(
                out=vt32[:, jt * 512:(jt + 1) * 512], in_=pv[0:64, 0:512]
            )

        # ---- u_all[p=(i2,b), pr] = U[2pr+i2, b] and u_mix (U_maxch -
        # U_reluch, the bias for the relu window), both built directly in the
        # bias layout via stride-2-column selector matmuls (a DRAM scatter
        # roundtrip here costs ~250us of full-pipeline stall)
        u_all = singles.tile([128, NPAIR], f32)
        u_mix = singles.tile([128, NPAIR], f32)
        for dst, negch in ((u_all, -1), (u_mix, RELUCH)):
            up = psum.tile([128, 1024], f32, tag="ps")
            for i2 in range(2):
                for ch in range(NCHUNK):
                    mv = mt_bf[ch][:, 0:ROWS].rearrange(
                        "p (pr two) -> p two pr", two=2)
                    nc.tensor.matmul(
                        up[i2 * 64:(i2 + 1) * 64, 0:NPAIR],
                        s_neg if ch == negch else s_sel,
                        mv[:, i2:i2 + 1, :],
                        start=(ch == 0),
                        stop=(ch == NCHUNK - 1),
                        skip_group_check=True,
                    )
            nc.scalar.copy(out=dst, in_=up[:, 0:NPAIR])

        # ---- per-(i,b) row partial sums: col = pr*3 + window ----------------
        psbuf = singles.tile([128, NPAIR * 3], f32)

        # ---- persistent column-sum accumulators (live across the pr loop);
        # both packed into one [128, 1024] psum tile: window A0 sums on
        # partitions 0-63, window A1 on 64-127 (matmul tile_position derives
        # from out.base_partition)
        colt = colp.tile([128, 1024], f32)
        colacc = [colt[0:64, :], colt[64:128, :]]

        # ---- main loop ------------------------------------------------------
        # windows: A0 = [0,1024) (diag+gap1a, step mask, col sums),
        #          A1 = [1024,2048) (gaps, col sums, relu chunk),
        #          B  = [2048,2560) (gap 4, row sums only)
        for pr in range(NPAIR):
            i0 = 2 * pr
            sk = min((i0 // 128) * 128, 384)
            # -- produce pairwise tiles for both rows of the pair ------------
            rhs = {}  # (i2, window) -> list of (tile, joff) rhs sources
            for i2 in range(2):
                i = i0 + i2
                # ch0/ch1: TS max over [sk, 2560), merged by a TT add (TT is
                # 2x fp16 mode; scalar_tensor_tensor measured 1x -> avoided)
                ad0 = adp_s.tile([128, JDOM], fp16, tag="ad0")
                nc.vector.tensor_scalar(
                    out=ad0[:, sk:JDOM],
                    in0=mt_bf[0][:, sk:JDOM],
                    scalar1=mt_f32[0][:, i:i + 1],
                    scalar2=None,
                    op0=Alu.max,
                )
                ad1 = adp_s.tile([128, JDOM], fp16, tag="ad1")
                nc.vector.tensor_scalar(
                    out=ad1[:, sk:JDOM],
                    in0=mt_bf[1][:, sk:JDOM],
                    scalar1=mt_f32[1][:, i:i + 1],
                    scalar2=None,
                    op0=Alu.max,
                )
                m01 = adp.tile([128, JDOM], fp16, tag="m01")
                nc.vector.tensor_tensor(
                    out=m01[:, sk:JDOM],
                    in0=ad0[:, sk:JDOM],
                    in1=ad1[:, sk:JDOM],
                    op=Alu.add,
                )
                # ch2: TS max over [sk, 2560)
                ad2 = adp.tile([128, JDOM], fp16, tag="ad2")
                nc.vector.tensor_scalar(
                    out=ad2[:, sk:JDOM],
                    in0=mt_bf[2][:, sk:JDOM],
                    scalar1=mt_f32[2][:, i:i + 1],
                    scalar2=None,
                    op0=Alu.max,
                )
                # ch3: TS max on A0 + B windows; ScalarE relu on A1
                ad3 = adp.tile([128, JDOM], fp16, tag="ad3")
                nc.vector.tensor_scalar(
                    out=ad3[:, sk:1024],
                    in0=mt_bf[3][:, sk:1024],
                    scalar1=mt_f32[3][:, i:i + 1],
                    scalar2=None,
                    op0=Alu.max,
                )
                nc.vector.tensor_scalar(
                    out=ad3[:, 2048:JDOM],
                    in0=mt_bf[3][:, 2048:JDOM],
                    scalar1=mt_f32[3][:, i:i + 1],
                    scalar2=None,
                    op0=Alu.max,
                )
                r3 = escr_p.tile([128, 1024], fp16, tag="r3")
                nc.scalar.activation(
                    out=r3,
                    in_=mt_bf[RELUCH][:, 1024:2048],
                    func=Act.Relu,
                    scale=1.0,
                    bias=mtn_f32[:, i:i + 1],
                )
                rhs[(i2, 0)] = [(m01, 0), (ad2, 0), (ad3, 0)]
                rhs[(i2, 1)] = [(m01, 0), (ad2, 0), (r3, 1024)]
                rhs[(i2, 2)] = [(m01, 0), (ad2, 0), (ad3, 0)]

            # -- windows -----------------------------------------------------
            for w, (j0, j1) in enumerate([(0, 1024), (1024, 2048), (2048, JDOM)]):
                jb0 = j0 + (sk if w == 0 else 0)
                dps = psum.tile([128, 1024], f32, tag="ps")
                # bank-aligned 512-col slices of [jb0, j1)
                js_chunks = []
                s = jb0
                while s < j1:
                    e = min((s // 512 + 1) * 512, j1)
                    js_chunks.append((s, e))
                    s = e
                # main selector matmuls
                for i2 in range(2):
                    for (js0, js1) in js_chunks:
                        for ri, (rt, roff) in enumerate(rhs[(i2, w)]):
                            nc.tensor.matmul(
                                dps[i2 * 64:(i2 + 1) * 64,
                                    js0 - j0:js1 - j0],
                                s_sel,
                                rt[:, js0 - roff:js1 - roff],
                                start=(ri == 0),
                                stop=False,
                                skip_group_check=True,
                            )
                # -V/2 correction
                for (js0, js1) in js_chunks:
                    nc.tensor.matmul(
                        dps[:, js0 - j0:js1 - j0],
                        e2_r,
                        vt32[:, js0:js1],
                        start=False,
                        stop=True,
                        skip_group_check=True,
                    )
                # step mask on A0: +25 for j <= i (covers the self term)
                if w == 0:
                    q = i0 - sk          # 0..126
                    nm = 128
                    off = 128 - q
                    nc.tensor.matmul(
                        dps[:, sk:sk + nm],
                        bw_sb,
                        stepb[:, off:off + nm],
                        start=False,
                        stop=True,
                        skip_group_check=True,
                    )
                # exp(-2P + bias), row sums via accum_out
                escr = escr_p.tile([128, 1024], fp16, tag="escr")
                nc.scalar.activation(
                    out=escr[:, jb0 - j0:j1 - j0],
                    in_=dps[:, jb0 - j0:j1 - j0],
                    func=Act.Exp,
                    scale=-2.0,
                    bias=(u_mix if w == 1 else u_all)[:, pr:pr + 1],
                    accum_out=psbuf[:, pr * 3 + w:pr * 3 + w + 1],
                )
                # column sums (A windows only): accumulate over all pr.
                # NOTE: start=True zeroes the whole 2KB psum bank, so each
                # bank of colacc must see exactly one start (its first MM).
                if w < 2:
                    if w == 0:
                        # bank 0: only [sk, 512) is valid escr this pr; cols
                        # below sk were completed by earlier prs
                        regions = [(sk, 512), (512, 1024)]
                    else:
                        regions = [(0, 512), (512, 1024)]
                    for (c0, c1) in regions:
                        nc.tensor.matmul(
                            colacc[w][:, c0:c1],
                            s_colw,
                            escr[:, c0:c1],
                            start=(pr == 0),
                            stop=(pr == NPAIR - 1),
                            skip_group_check=True,
                        )

        # ---- finish: row part ----------------------------------------------
        red = singles.tile([128, NPAIR], f32)
        tmp = singles.tile([128, NPAIR], f32)
        pv3 = psbuf.rearrange("p (c w) -> p c w", w=3)
        nc.vector.tensor_tensor(
            out=tmp, in0=pv3[:, :, 0], in1=pv3[:, :, 1], op=Alu.add
        )
        nc.vector.tensor_tensor(
            out=red, in0=tmp, in1=pv3[:, :, 2], op=Alu.add
        )
        # red[:, pr]: partition = i2*64 + b. Transpose 128-blocks so the DMA
        # descriptors are contiguous 256B runs.
        o_v = or_d.rearrange("(pr i2) b -> pr i2 b", i2=2)
        for blk in range(NPAIR // 128):
            pt = psum.tile([128, 1024], f32, tag="ps")
            nc.tensor.transpose(
                pt[:, 0:128], red[:, blk * 128:(blk + 1) * 128], ident
            )
            ot = xin.tile([128, 128], f32, tag="otile")
            nc.scalar.copy(out=ot, in_=pt[:, 0:128])
            ot_v = ot.rearrange("q (i2 b) -> q i2 b", i2=2)
            nc.sync.dma_start(out=o_v[blk * 128:(blk + 1) * 128], in_=ot_v)

        # ---- finish: column part (partition-aligned copy, remap in the DMA:
        # partitions (w b), free j -> out_col[b, w*1024 + j])
        col_sb = singles.tile([128, 1024], f32)
        nc.scalar.copy(out=col_sb, in_=colt)
        nc.sync.dma_start(out=oc_d[:, 0:1024], in_=col_sb[0:64, :])
        nc.sync.dma_start(out=oc_d[:, 1024:2048], in_=col_sb[64:128, :])

    nc.compile()
    return nc


def _get_program():
    if "nc" not in _CACHE:
        _CACHE["nc"] = _build_program()
    return _CACHE["nc"]


def _host_consts():
    s_sel = (np.arange(128)[:, None] // 2 == np.arange(B)[None, :]).astype(
        np.float16
    )
    s_neg = -s_sel
    s_colw = (np.arange(128)[:, None] % 64 == np.arange(B)[None, :]).astype(
        np.float16
    )
    ident = np.eye(128, dtype=np.float32)
    e2_neg = (-0.5 * (np.arange(64)[:, None] == (np.arange(128)[None, :] % 64))
              ).astype(np.float32)
    bw_mask = np.zeros((32, 128), dtype=np.float16)
    bw_mask[0, :64] = 25.0
    bw_mask[1, 64:] = 25.0
    stepb = np.zeros((32, 512), dtype=np.float16)
    stepb[0, : 128 + 1] = 1.0
    stepb[1, : 129 + 1] = 1.0
    return s_sel, s_neg, s_colw, ident, e2_neg, bw_mask, stepb


def _host_inputs(x, T):
    x = np.ascontiguousarray(x, dtype=np.float32)
    # permute T columns: q = ch*128 + b*2 + e  <->  (b, c=2ch+e)
    t_mat = np.ascontiguousarray(
        T.reshape(F, B, NCHUNK, 2).transpose(0, 2, 1, 3).reshape(F, BC),
        dtype=np.float32,
    )
    return x, t_mat


TRACE = bool(int(os.environ.get("KERNEL_TRACE", "0")))
LAST_RESULTS = None


def _make_ntff_hook():
    # the image's antenv lacks axon_hooks, but the injected libaxon_pjrt.so
    # carries the NTFF profile C ABI — drive it via ctypes directly
    import contextlib
    import ctypes

    so_path = "/opt/axon/libaxon_pjrt.so"
    if not os.path.exists(so_path):
        return None
    lib = ctypes.CDLL(so_path)
    if not hasattr(lib, "axon_start_nrt_profile"):
        return None
    lib.axon_start_nrt_profile.argtypes = [
        ctypes.POINTER(ctypes.c_int64),
        ctypes.c_size_t,
    ]
    lib.axon_start_nrt_profile.restype = ctypes.c_int64
    lib.axon_stop_nrt_profile.argtypes = [ctypes.c_char_p]
    lib.axon_stop_nrt_profile.restype = ctypes.c_int64

    @contextlib.contextmanager
    def _hook(output_dir, device_ids):
        import jax

        jax.devices()
        if device_ids:
            ids = (ctypes.c_int64 * len(device_ids))(*device_ids)
            rc = lib.axon_start_nrt_profile(ids, len(device_ids))
        else:
            rc = lib.axon_start_nrt_profile(None, 0)
        if rc != 0:
            raise RuntimeError(f"axon_start_nrt_profile rc={rc}")
        try:
            yield
        finally:
            n = lib.axon_stop_nrt_profile(str(output_dir).encode())
            print(f"profile: {n} file(s) written to {output_dir}")

    return _hook


def _ensure_axon_hook_stub():
    import importlib
    import sys
    import types

    try:
        importlib.import_module("antenv.axon_hooks")
    except ModuleNotFoundError:
        stub = types.ModuleType("antenv.axon_hooks")
        stub.get_axon_ntff_profile_hook = _make_ntff_hook
        sys.modules["antenv.axon_hooks"] = stub


def kernel(x: np.ndarray, T: np.ndarray) -> np.ndarray:
    global LAST_RESULTS
    _ensure_axon_hook_stub()
    from concourse.bass_utils import run_bass_kernel_spmd

    nc = _get_program()
    x, t_mat = _host_inputs(x, T)
    s_sel, s_neg, s_colw, ident, e2_neg, bw_mask, stepb = _host_consts()

    in_maps = []
    for k in range(NCORES):
        x_rot = np.roll(x, -ROWS * k, axis=0)[:JDOM] if k else x[:JDOM]
        in_maps.append(
            {"x_rot": np.ascontiguousarray(x_rot), "t_mat": t_mat,
             "s_sel": s_sel, "s_neg": s_neg, "s_colw": s_colw,
             "ident": ident, "e2_neg": e2_neg, "bw_mask": bw_mask,
             "stepb": stepb}
        )

    res = run_bass_kernel_spmd(
        nc, in_maps, core_ids=list(range(NCORES)), trace=TRACE
    )
    LAST_RESULTS = res
    out = np.zeros((N, B), dtype=np.float64)
    for k in range(NCORES):
        out[k * ROWS:(k + 1) * ROWS] += res.results[k]["out_row"].astype(
            np.float64
        )
        colp = res.results[k]["out_col"].astype(np.float64).T  # [COLW, B]
        gidx = (k * ROWS + np.arange(COLW)) % N
        np.add.at(out, gidx, colp)
    return np.concatenate([x, out.astype(np.float32)], axis=1)
